# revision 8
# baseline (speedup 1.0000x reference)
"""MoELoRALinear Trainium2 kernel (8-core data-parallel, Bass/Tile).

Math (per token t, out feature o):
    out[t,o] = x[t,:] @ base_w[o,:] + base_b[o]
             + sum_e softmax_e(x[t,:] @ router_w[e,:]) * SCALE
               * sum_r (x[t,:] @ A[e,r,:]) * B[e,o,r]

Strategy (v2: mixed bf16/fp8-DoubleRow, ~?us on HW; v1 bf16 was 151.5us):
  - 8192 tokens sharded 8 ways (1024 tokens/core); weights replicated.
  - fp8 e4m3 DoubleRow (2 fp8 MACs/PE/cycle) on the error-tolerant parts:
    * Y phase ([A;router].T @ x over all 16 k-chunks): 4096 cyc vs 16384.
    * close matmul (gated-proj + bias): [128,2,*] DoubleRow with a zero
      second k-tile: 256 cyc/group vs 512.
    * first F8C=4 of 16 base k-chunks (x8(1x) . w8(64x)): rel-err grows
      as 3.4e-2*sqrt(F8C/16); F8C=4 simulates to 1.64e-2 < 2e-2 gate.
  - Unified PSUM scale 64: w images *64, w1t *16, bcat rows *8 (=64/16*SCALE),
    bias row *64; exp reads logits with scale=1/16; output cast scales 1/64.
  - Everything else identical in spirit to v1: gating with no PE transposes,
    single-buf ypsum chain rotation hidden behind bf16 base waves, two DMA
    rings with ramp-ordered pieces, warmup matmuls over the NEFF prologue,
    final tile in two 256-wide half-groups.
"""

import os

import numpy as np

import concourse.bacc as bacc
import concourse.bass as bass
import concourse.mybir as mybir
from concourse.bass_utils import run_bass_kernel_spmd
from concourse.tile import TileContext

SCALE = 16.0 / 8.0  # alpha / r

N_CORES = 8
TOK = 8192  # 4 * 2048 tokens total
TPC = TOK // N_CORES  # tokens per core = 1024
D = 2048  # in features
O = 2048  # out features
E = 4
R = 8
ER = E * R  # 32
J = ER + E  # 36: rank-proj rows + router rows
JP = 48  # J padded to mult-of-16 (dual-fp8 ldweights column restriction)
DC = D // 128  # 16 contraction chunks
F8C = 4  # k-chunks 0..3 of the base matmul run in fp8 DoubleRow
NBF = DC - F8C  # 12 bf16 k-chunks (dc 4..15)
OCW = 512  # out-feature chunk width (one PSUM bank)
OC = O // OCW  # 4
TC = TPC // 128  # 8 token chunks per core
WAVE = 4  # wave-A token chunks

SW = 64.0  # base-w / group scale
S1 = 16.0  # w1 ([A; router]) scale

F32 = mybir.dt.float32
BF16 = mybir.dt.bfloat16
FP8 = mybir.dt.float8e4
DR = mybir.MatmulPerfMode.DoubleRow

# Results of the last device run (for test harness inspection).
last_run_info: dict = {}

_cached = None


def _build_program():
    nc = bacc.Bacc()

    x8_d = nc.declare_dram_parameter("x8", [128, DC, TPC], FP8, isOutput=False)
    xb_d = nc.declare_dram_parameter("xb", [128, NBF, TPC], BF16, isOutput=False)
    w8_d = nc.declare_dram_parameter("w8", [OC, 128, F8C // 2, OCW, 2], FP8, isOutput=False)
    wt_d = nc.declare_dram_parameter("wt", [OC, 128, NBF * OCW], BF16, isOutput=False)
    w1t_d = nc.declare_dram_parameter("w1t", [128, DC, JP], FP8, isOutput=False)
    bcat_d = nc.declare_dram_parameter("bcat", [128, O, 2], FP8, isOutput=False)
    cst_d = nc.declare_dram_parameter("cst", [E, ER + 4], BF16, isOutput=False)
    out_d = nc.declare_dram_parameter("out", [OC, TC, 128, OCW], BF16, isOutput=True)

    MUL = mybir.AluOpType.mult

    with TileContext(nc) as tc:
        with (
            tc.tile_pool(name="cpool", bufs=1) as cpool,
            tc.tile_pool(name="wpool", bufs=4) as wpool,
            tc.tile_pool(name="opool", bufs=6) as opool,
            tc.tile_pool(name="mpsum", bufs=7, space="PSUM") as mpsum,
            tc.tile_pool(name="ypsum", bufs=1, space="PSUM") as ypsum,
        ):
            # Ring plan (ramp-ordered, 3 rings): sync = x8 (the Y ladder,
            # first pair split in token-halves for earliest start) + outputs;
            # gpsimd = bf16 x chunks dc4..11; scalar = w1t8 first, then
            # w8[0], wbf[0] k-pieces, x tail dc12..15, cst/bcat8, oc1..3.
            w1tr = cpool.tile([128, DC, JP], FP8)

            x8 = cpool.tile([128, DC, TPC], FP8)
            for h in range(2):
                nc.sync.dma_start(
                    out=x8[:, 0:2, h * 512 : (h + 1) * 512],
                    in_=x8_d[:, 0:2, h * 512 : (h + 1) * 512],
                )
            for a, b in ((2, 4), (4, 6), (6, 8), (8, 12), (12, 16)):
                nc.sync.dma_start(out=x8[:, a:b, :], in_=x8_d[:, a:b, :])

            XSPL = 8  # bf16 chunks dc4..11 on sync, dc12..15 on scalar
            xa = cpool.tile([128, XSPL * TPC], BF16)
            xb = cpool.tile([128, (NBF - XSPL) * TPC], BF16)

            def xsl(dc, a, b):
                # dc in 4..15 (bf16 chunks)
                i = dc - F8C
                if i < XSPL:
                    return xa[:, i * TPC + a : i * TPC + b]
                return xb[:, (i - XSPL) * TPC + a : (i - XSPL) * TPC + b]

            w8s = {
                oc: wpool.tile([128, F8C // 2, OCW, 2], FP8, name=f"w8r{oc}", tag="w8r")
                for oc in range(OC)
            }
            wts = {
                oc: wpool.tile([128, NBF * OCW], BF16, name=f"wtr{oc}", tag="wtr")
                for oc in range(OC)
            }
            nc.scalar.dma_start(out=w1tr, in_=w1t_d[:, :, :])
            nc.scalar.dma_start(out=w8s[0], in_=w8_d[0])
            for k in range(4):
                nc.scalar.dma_start(
                    out=wts[0][:, k * 3 * OCW : (k + 1) * 3 * OCW],
                    in_=wt_d[0, :, k * 3 * OCW : (k + 1) * 3 * OCW],
                )
            for dp in range(2):
                nc.gpsimd.dma_start(
                    out=xa[:, dp * TPC : (dp + 1) * TPC],
                    in_=xb_d[:, dp : dp + 1, :],
                )
            for dp in range(2, 8, 2):
                nc.gpsimd.dma_start(
                    out=xa[:, dp * TPC : (dp + 2) * TPC],
                    in_=xb_d[:, dp : dp + 2, :],
                )
            nc.scalar.dma_start(out=xb[:, 0 : 2 * TPC], in_=xb_d[:, 8:10, :])
            nc.scalar.dma_start(out=xb[:, 2 * TPC : 4 * TPC], in_=xb_d[:, 10:12, :])
            cstr = cpool.tile([E, ER + 4], BF16)
            nc.scalar.dma_start(out=cstr, in_=cst_d[:, :])
            bcatr = cpool.tile([128, O, 2], FP8)
            nc.scalar.dma_start(out=bcatr, in_=bcat_d[:, :, :])
            for oc in range(1, OC):
                nc.scalar.dma_start(out=w8s[oc], in_=w8_d[oc])
                for h in range(2):
                    nc.scalar.dma_start(
                        out=wts[oc][:, h * 6 * OCW : (h + 1) * 6 * OCW],
                        in_=wt_d[oc, :, h * 6 * OCW : (h + 1) * 6 * OCW],
                    )

            # Warmup: ramp the PE p-state on a zero tile while the first
            # x8 pair is still in flight.
            warm_sb = cpool.tile([128, OCW], BF16)
            nc.vector.memset(warm_sb, 0.0)
            # Close lhsT: fp8 [128, 2, TPC]; k-tile 0 = 32 gated-proj rows +
            # ones row + zeros, k-tile 1 all zero (DoubleRow padding).
            vwtr = cpool.tile([128, 2, TPC], FP8)
            nc.vector.memset(vwtr, 0.0)
            nc.vector.memset(vwtr[ER : ER + 1, 0, :], 1.0)
            warmps = ypsum.tile([128, OCW], F32, name="warmps", tag="yb")
            for _ in range(8):
                nc.tensor.matmul(
                    warmps, lhsT=warm_sb[:, 0:128], rhs=warm_sb, start=True, stop=True
                )

            # --- Y phase: YT[j, tok] over all 16 k-chunks as fp8 DoubleRow
            # pairs, interleaved with wave-A fp8 base pairs in arrival order.
            warmf = mpsum.tile([128, OCW], F32, name="warmf", tag="ps")
            ytps = [
                ypsum.tile([JP, 512], F32, name="ytps0", tag="yb"),
                mpsum.tile([JP, 512], F32, name="ytps1", tag="ps"),
            ]
            psA = {
                t: mpsum.tile([128, OCW], F32, name=f"ps0_{t}", tag="ps")
                for t in range(WAVE)
            }

            def bf_part(ps, oc, t, dcs):
                for dc in dcs:
                    nc.tensor.matmul(
                        ps,
                        lhsT=xsl(dc, t * 128, (t + 1) * 128),
                        rhs=wts[oc][:, (dc - F8C) * OCW : (dc - F8C + 1) * OCW],
                        start=False,
                        stop=False,
                    )

            def waveA_bf(dcs):
                for dc in dcs:
                    for t in range(WAVE):
                        bf_part(psA[t], 0, t, [dc])
            for kp in range(8):
                for th in range(2):
                    nc.tensor.matmul(
                        ytps[th],
                        lhsT=w1tr[:, 2 * kp : 2 * kp + 2, :],
                        rhs=x8[:, 2 * kp : 2 * kp + 2, th * 512 : (th + 1) * 512],
                        start=(kp == 0),
                        stop=(kp == 7),
                        perf_mode=DR,
                    )
                if kp < 2:
                    for _ in range(2):
                        nc.tensor.matmul(
                            warmf,
                            lhsT=warm_sb[:, 0:128],
                            rhs=warm_sb,
                            start=True,
                            stop=True,
                        )
                if kp in (2, 3):
                    p = kp - 2
                    for t in range(WAVE):
                        nc.tensor.matmul(
                            psA[t],
                            lhsT=x8[:, 2 * p : 2 * p + 2, t * 128 : (t + 1) * 128],
                            rhs=w8s[0][:, p, :, :].transpose([0, 2, 1]),
                            start=(p == 0),
                            stop=False,
                            perf_mode=DR,
                        )
                if kp >= 4:
                    waveA_bf([kp])

            # Close-DR k-tile1 carries the dc3 w-residual correction:
            # lhsT tile1 = x8[dc3], rhs tile1 = wlo (= 64*w_dc3 - e4m3(64*w_dc3)).
            nc.vector.tensor_copy(vwtr[:, 1:2, :], x8[:, 3:4, :])

            # --- Gating chain (no PE transposes).
            y_sb = cpool.tile([ER, TPC], BF16)
            u_sb = cpool.tile([E, TPC], BF16)
            r_sb = cpool.tile([E, TPC], F32)
            g_sb = cpool.tile([E, TPC], BF16)
            for th in range(2):
                nc.vector.tensor_copy(
                    y_sb[:, th * 512 : (th + 1) * 512], ytps[th][0:ER, :]
                )
                nc.scalar.activation(
                    u_sb[:, th * 512 : (th + 1) * 512],
                    ytps[th][ER:J, :],
                    mybir.ActivationFunctionType.Exp,
                    scale=1.0 / S1,
                )

            def sums_mm(th):
                s = ypsum.tile([E, 512], F32, name=f"sums{th}", tag="yb")
                nc.tensor.matmul(
                    s,
                    lhsT=cstr[0:E, ER : ER + 4],
                    rhs=u_sb[:, th * 512 : (th + 1) * 512],
                    start=True,
                    stop=True,
                )
                return s

            def recip_g(th, s):
                nc.vector.reciprocal_approx_fast(
                    out=r_sb[:, th * 512 : (th + 1) * 512], in_=s
                )
                nc.vector.tensor_tensor(
                    g_sb[:, th * 512 : (th + 1) * 512],
                    u_sb[:, th * 512 : (th + 1) * 512],
                    r_sb[:, th * 512 : (th + 1) * 512],
                    op=MUL,
                )

            def gb_mm(th):
                gb = ypsum.tile([ER, 512], F32, name=f"gb{th}", tag="yb")
                nc.tensor.matmul(
                    gb,
                    lhsT=cstr[0:E, 0:ER],
                    rhs=g_sb[:, th * 512 : (th + 1) * 512],
                    start=True,
                    stop=True,
                )
                return gb

            def vw_tt(th, gb):
                nc.vector.tensor_tensor(
                    vwtr[0:ER, 0, th * 512 : (th + 1) * 512],
                    y_sb[:, th * 512 : (th + 1) * 512],
                    gb,
                    op=MUL,
                )

            # Chain hops hidden behind the wave-A bf16 k-chunks (each pair of
            # dc's = ~1.7us of base matmuls).
            waveA_bf([8, 9])
            s0 = sums_mm(0)
            recip_g(0, s0)
            waveA_bf([10, 11])
            s1 = sums_mm(1)
            recip_g(1, s1)
            waveA_bf([12, 13])
            gb0 = gb_mm(0)
            vw_tt(0, gb0)
            waveA_bf([14, 15])
            gb1 = gb_mm(1)
            vw_tt(1, gb1)

            def open_group(oc, t, name):
                ps = mpsum.tile([128, OCW], F32, name=name, tag="ps")
                for p in range(F8C // 2):
                    nc.tensor.matmul(
                        ps,
                        lhsT=x8[:, 2 * p : 2 * p + 2, t * 128 : (t + 1) * 128],
                        rhs=w8s[oc][:, p, :, :].transpose([0, 2, 1]),
                        start=(p == 0),
                        stop=False,
                        perf_mode=DR,
                    )
                bf_part(ps, oc, t, range(F8C, DC))
                return ps

            # --- Close: fused LoRA-up + bias DoubleRow, scaled cast, DMA out.
            def close_group(ps, oc, t, last=False):
                nc.tensor.matmul(
                    ps,
                    lhsT=vwtr[:, :, t * 128 : (t + 1) * 128],
                    rhs=bcatr[:, oc * OCW : (oc + 1) * OCW, :].transpose([0, 2, 1]),
                    start=False,
                    stop=True,
                    perf_mode=DR,
                )
                ot = opool.tile([128, OCW], BF16, tag="ot")
                if last:
                    h = OCW // 2
                    nc.vector.tensor_scalar_mul(ot[:, 0:h], ps[:, 0:h], 1.0 / SW)
                    nc.scalar.activation(
                        ot[:, h:OCW],
                        ps[:, h:OCW],
                        mybir.ActivationFunctionType.Copy,
                        scale=1.0 / SW,
                    )
                    nc.sync.dma_start(out=out_d[oc, t, :, 0:h], in_=ot[:, 0:h])
                    nc.sync.dma_start(
                        out=out_d[oc, t, :, h:OCW], in_=ot[:, h:OCW]
                    )
                else:
                    nc.vector.tensor_scalar_mul(ot, ps, 1.0 / SW)
                    nc.sync.dma_start(out=out_d[oc, t], in_=ot)

            for t in range(WAVE):
                close_group(psA[t], 0, t)
            for t in range(WAVE, TC):
                close_group(open_group(0, t, f"ps0_{t}"), 0, t)
            for oc in range(1, OC):
                for t in range(TC):
                    if oc == OC - 1 and t == TC - 1:
                        continue
                    close_group(open_group(oc, t, f"ps{oc}_{t}"), oc, t)

            # Final tile in two 256-wide half-groups so the post-last-matmul
            # tail is half a cast+DMA.
            oc, t = OC - 1, TC - 1
            HW_ = OCW // 2
            for h in range(2):
                ph = mpsum.tile([128, HW_], F32, name=f"psl{h}", tag="ps")
                for p in range(F8C // 2):
                    nc.tensor.matmul(
                        ph,
                        lhsT=x8[:, 2 * p : 2 * p + 2, t * 128 : (t + 1) * 128],
                        rhs=w8s[oc][
                            :, p, h * HW_ : (h + 1) * HW_, :
                        ].transpose([0, 2, 1]),
                        start=(p == 0),
                        stop=False,
                        perf_mode=DR,
                    )
                for dc in range(F8C, DC):
                    nc.tensor.matmul(
                        ph,
                        lhsT=xsl(dc, t * 128, (t + 1) * 128),
                        rhs=wts[oc][
                            :,
                            (dc - F8C) * OCW + h * HW_ : (dc - F8C) * OCW
                            + (h + 1) * HW_,
                        ],
                        start=False,
                        stop=False,
                    )
                nc.tensor.matmul(
                    ph,
                    lhsT=vwtr[:, :, t * 128 : (t + 1) * 128],
                    rhs=bcatr[
                        :, oc * OCW + h * HW_ : oc * OCW + (h + 1) * HW_, :
                    ].transpose([0, 2, 1]),
                    start=False,
                    stop=True,
                    perf_mode=DR,
                )
                ot = opool.tile([128, HW_], BF16, tag="ot")
                nc.vector.tensor_scalar_mul(ot, ph, 1.0 / SW)
                nc.sync.dma_start(
                    out=out_d[oc, t, :, h * HW_ : (h + 1) * HW_], in_=ot
                )

    nc.compile()
    return nc


def _prep_inputs(x, base_w, base_b, A, B, router_w):
    """Host-side layout prep: per-partition-contiguous DMA images."""
    import ml_dtypes

    bf16 = ml_dtypes.bfloat16
    e4m3 = ml_dtypes.float8_e4m3fn

    def q8(v):
        return np.clip(np.asarray(v, np.float32), -240.0, 240.0).astype(e4m3)

    x2 = np.ascontiguousarray(x, dtype=np.float32).reshape(TOK, D)
    # x images: [core][p, dc, t] = x2[core*TPC + t, dc*128 + p]
    xv = x2.reshape(N_CORES, TPC, DC, 128).transpose(0, 3, 2, 1)
    xv = np.ascontiguousarray(xv)  # [core, 128, DC, TPC]
    x8t = q8(xv)  # all 16 chunks in fp8 (Y phase + base pairs)
    xbt = xv[:, :, F8C:, :].astype(bf16)  # dc 4..15 bf16

    # w images (*SW): [oc, p, dc, o] = 64*base_w[oc*512+o, dc*128+p]
    wv = (np.asarray(base_w, np.float32) * SW).reshape(OC, OCW, DC, 128)
    wv = np.ascontiguousarray(wv.transpose(0, 3, 2, 1))  # [OC, 128, DC, OCW]
    # interleaved pairs: w8t[oc, p, pair, o, i] = 64*w[oc*512+o, (2*pair+i)*128+p]
    w8t = q8(
        wv[:, :, 0:F8C, :]
        .reshape(OC, 128, F8C // 2, 2, OCW)
        .transpose(0, 1, 2, 4, 3)
    )
    wbt = np.ascontiguousarray(wv[:, :, F8C:, :]).reshape(
        OC, 128, NBF * OCW
    ).astype(bf16)

    # W1 = [A flattened to 32 rows; router_w 4 rows] over D, *S1, fp8
    W1 = (
        np.concatenate(
            [
                np.asarray(A, dtype=np.float32).reshape(ER, D),
                np.asarray(router_w, np.float32),
            ],
            axis=0,
        )
        * S1
    )  # [36, D]
    w1t8 = q8(
        np.ascontiguousarray(
            np.concatenate([W1, np.zeros((JP - J, D), np.float32)], axis=0)
            .reshape(JP, DC, 128)
            .transpose(2, 1, 0)
        )
    )  # [128, DC, JP]

    # w_lo residual for dc3: wlo[p, o] = 64*w[o, 3*128+p] - float(e4m3(same))
    w64 = np.asarray(base_w, np.float32) * SW  # [O, D]
    wsl = w64[:, 3 * 128 : 4 * 128]  # [O, 128]
    wlo8 = q8(np.ascontiguousarray((wsl - q8(wsl).astype(np.float32)).T))  # [128, O]

    # bcat rows 0..31: (SW/S1)*SCALE * B^T; row 32: SW*base_b; rest zero.
    bc = np.zeros((128, O), np.float32)
    bc[0:ER] = (
        np.asarray(B, dtype=np.float32).transpose(0, 2, 1).reshape(ER, O)
        * (SW / S1)
        * SCALE
    )
    bc[ER] = np.asarray(base_b, dtype=np.float32) * SW
    bc8 = q8(bc)
    # interleave close rhs k-tiles: [p, o, {bcat, wlo}]
    bcil = np.stack([bc8, wlo8], axis=-1)  # [128, O, 2] fp8

    # cst[:, :32] = per-expert expansion (E8); cst[:, 32:36] = ones (sums
    # replicate s onto 4 rows; recip gives 1/s; SCALE lives in bcat).
    cst = np.zeros((E, ER + 4), np.float32)
    for e in range(E):
        cst[e, e * R : (e + 1) * R] = 1.0
    cst[:, ER : ER + 4] = 1.0
    cst = cst.astype(bf16)

    return x8t, xbt, w8t, wbt, w1t8, bcil, cst


def kernel(x, base_w, base_b, A, B, router_w):
    global _cached
    if _cached is None:
        _cached = _build_program()
    nc = _cached

    x8t, xbt, w8t, wbt, w1t8, bcil, cst = _prep_inputs(
        x, base_w, base_b, A, B, router_w
    )

    in_maps = [
        {
            "x8": x8t[c],
            "xb": xbt[c],
            "w8": w8t,
            "wt": wbt,
            "w1t": w1t8,
            "bcat": bcil,
            "cst": cst,
        }
        for c in range(N_CORES)
    ]
    core_ids = list(range(N_CORES))

    profile = os.environ.get("KERNEL_PROFILE", "0") == "1"
    res = run_bass_kernel_spmd(nc, in_maps, core_ids, trace=profile)

    last_run_info.clear()
    last_run_info["exec_time_ns"] = res.exec_time_ns
    last_run_info["mean_exec_time_ns"] = res.mean_exec_time_ns
    last_run_info["instructions_and_trace"] = res.instructions_and_trace
    last_run_info["profile_json"] = res.profile_json

    # out[core] shape [OC, TC, 128, OCW] bf16 -> tokens x features fp32
    full = np.empty((TOK, O), dtype=np.float32)
    for c in range(N_CORES):
        buf = res.results[c]["out"].astype(np.float32)  # [OC, TC, 128, OCW]
        full[c * TPC : (c + 1) * TPC] = buf.transpose(1, 2, 0, 3).reshape(TPC, O)
    return full.reshape(4, 2048, 2048)
